# revision 44
# baseline (speedup 1.0000x reference)
"""BFP activation quantization kernel for 8 TRN2 NeuronCores (v5, bf16 domain).

Reference semantics (per (n,h,w) block over C=256 channels):
    max_abs = max_c |x|
    e such that max_abs = m * 2^e, m in [0.5, 1)   (frexp)
    delta = 2^(e-3)
    q = trunc(x / delta) * delta

Sharding: pure data-parallel over N (batch): 64 images -> 8 per core.

Key fact: the BFP output keeps at most sign + exponent + 2 mantissa bits, and
the k*delta grid points are bf16-representable, so bf16-TRUNCATION of x (high
16 bits of each fp32 word) commutes exactly with the quantizer, and the
block-max exponent equals the max of per-element exponent fields. The whole
per-element pipeline therefore runs at 16-bit DVE rates, and the GPSIMD
cross-partition reduce sees exponent-only int16 data at half the columns.

Per-image chain (layout [c -> partitions, hw -> free], F = 3136):
  I1  x16 = high_halves(x) | 1          int16, odd mantissa (tie-safe cvt)
  I2  exf = x16 & 0x7F80                exponent fields, positive int16
  I3  m   = max(exf_half0, exf_half1)   [128, F]
  I4  mx  = partition_all_reduce(max)   block-max exponent field eb, bcast
  I5  rn  = -eb                         = bits(-1/delta) as bf16
  I6  dd  = eb - 0x0100                 = bits(delta) as bf16
  I7  t   = x' * (-1/delta)             bf16 exact product; u = -t = x'/delta
  I8  p   = cvt_i16(Relu(-t - cm))      cm = 0.5 - 2^-25 -> trunc(u), u>0
  I9  n   = cvt_i16(Relu( t - cm))      -> trunc(|u|), u<0   (ScalarE)
  I10 w   = p - n  (as bf16)            signed trunc(u), |w| <= 7
  I11 q   = w * delta                   bf16, exact
  out: DMA with SWDGE bf16->f32 cast.

Tile lifetimes keep xt (the 25KB input tile) dead after I1 so the input DMA
can prefetch deeply (bufs=2 on the big pool).
"""

import sys

for _p in ("/opt/trn_rl_repo", "/root/.axon_site/_ro/trn_rl_repo"):
    if _p not in sys.path:
        sys.path.append(_p)

import numpy as np

N, C, H, W = 64, 256, 56, 56
HW = H * W  # 3136
NCORES = 8
NPC = N // NCORES  # images per core
F = 3136

DEFAULT_VARIANT = "full"

_cache = {}


def _build(R=1, variant=None):
    if variant is None:
        variant = DEFAULT_VARIANT
    key = ("nc", R, variant)
    if key in _cache:
        return _cache[key]

    import concourse.bacc as bacc
    import concourse.mybir as mybir
    import concourse.tile as tile
    from concourse import bass_isa

    dt = mybir.dt
    op = mybir.AluOpType

    nc = bacc.Bacc(
        "TRN2",
        target_bir_lowering=False,
        debug=False,
        enable_asserts=False,
        num_devices=NCORES,
    )
    x_d = nc.dram_tensor("x", [NPC, C, HW], dt.float32, kind="ExternalInput").ap()
    y_d = nc.dram_tensor("y", [NPC, C, HW], dt.float32, kind="ExternalOutput").ap()

    pools = {"bufs3": (2, 3, 3), "halfimg": (3, 4, 3)}.get(variant, (2, 3, 2))
    with tile.TileContext(nc) as tc:
        with (
            tc.tile_pool(name="big", bufs=pools[0]) as big,
            tc.tile_pool(name="deep", bufs=pools[1]) as deep,
            tc.tile_pool(name="midp", bufs=3) as midp,
            tc.tile_pool(name="qtp", bufs=3) as qtp,
            tc.tile_pool(name="small", bufs=pools[2]) as small,
            tc.tile_pool(name="consts", bufs=1) as consts,
        ):
            negcm = consts.tile([128, 1], dt.float32)  # -(0.5 - 2^-25)
            nc.vector.memset(negcm[:], -0.4999999701976776123046875)
            zerob = consts.tile([128, 1], dt.float32)
            nc.vector.memset(zerob[:], 0.0)
            if variant == "hwout":
                _hwout_body(nc, big, small, negcm, zerob, x_d, y_d, R)
            elif variant in ("full", "bufs3", "halfimg"):
                _v7_body(nc, big, deep, midp, qtp, small, negcm, x_d, y_d, R,
                         S=2 if variant == "halfimg" else 1)
            else:
                _v5_body(nc, big, small, negcm, x_d, y_d, R, variant)
    nc.compile()
    _cache[key] = nc
    return nc


def _v7_body(nc, big, deep, midp, qtp, small, negcm, x_d, y_d, R, S=1):
    """v8: 2-stage software pipeline. Iteration k emits: frontend(k)
    [DMA, I1, I2, I3, reduce], out-DMA(k-2) on gpsimd (input long ready),
    mid(k-1) [dd, rn, I7, ScalarE Relu pair], tail(k-2) [I10, I11].
    The DVE queue thus never waits on the GPSIMD reduce (hidden behind
    frontend(k)) nor on ScalarE (hidden behind frontend+mid of later
    sub-images).

    Slots: xt(input + m staged in its i16 view, dies at reduce),
    x16(x'|1 -> n16, 3-deep), pt(exf -> p16 -> w, 3-deep),
    qt(t -> q, 3-deep), mx(-> rn in-place), dd.
    """
    import concourse.mybir as mybir
    from concourse import bass_isa

    dt = mybir.dt
    op = mybir.AluOpType
    F2 = F // S
    pending_out = None   # (n, s, qt) awaiting its SWDGE out-DMA
    pending_mid = None   # (n, s, x16, pt, mx) awaiting mid stage
    pending_tail = None  # (n, s, x16, pt, dd, qt) awaiting tail stage

    def flush_out():
        nonlocal pending_out
        if pending_out is not None:
            pn, ps, pq = pending_out
            hs = slice(ps * F2, (ps + 1) * F2)
            nc.gpsimd.dma_start(out=y_d[pn, 0:128, hs], in_=pq[:, 0:F2])
            nc.gpsimd.dma_start(
                out=y_d[pn, 128:256, hs], in_=pq[:, F2 : 2 * F2]
            )
            pending_out = None

    def emit_mid():
        nonlocal pending_mid, pending_tail
        if pending_mid is None:
            return
        bn, bs, x16, pt, mx = pending_mid
        pending_mid = None
        xb = x16[:].bitcast(dt.bfloat16)
        # dd = bits(delta) = eb - 0x0100
        dd = small.tile([128, F2], dt.int16, tag="dd")
        nc.vector.tensor_scalar(
            out=dd[:], in0=mx[:], scalar1=-0x0100, scalar2=None, op0=op.add
        )
        # rn = bits(-1/delta) = -eb (in-place over mx, AFTER dd)
        rn = mx
        nc.vector.tensor_scalar(
            out=rn[:], in0=mx[:], scalar1=-1, scalar2=None, op0=op.mult
        )
        # I7: t = x' * (-1/delta) -> qt
        qt = qtp.tile([128, 2 * F2], dt.bfloat16, tag="qt")
        rnb = rn[:].bitcast(dt.bfloat16)[:, None, :].broadcast_to([128, 2, F2])
        nc.vector.tensor_tensor(
            out=qt[:].rearrange("p (r f) -> p r f", r=2),
            in0=xb.rearrange("p (r f) -> p r f", r=2),
            in1=rnb, op=op.mult,
        )
        # I8/I9: Relu pair + RN cvt (ScalarE): p -> pt (exf dead),
        # n -> x16 (x' dead)
        nc.scalar.activation(
            out=pt[:], in_=qt[:], func=mybir.ActivationFunctionType.Relu,
            bias=negcm[:], scale=-1.0,
        )
        nc.scalar.activation(
            out=x16[:], in_=qt[:], func=mybir.ActivationFunctionType.Relu,
            bias=negcm[:], scale=1.0,
        )
        pending_tail = (bn, bs, x16, pt, dd, qt)

    def emit_tail():
        nonlocal pending_tail, pending_out
        if pending_tail is None:
            return
        bn, bs, x16, pt, dd, qt = pending_tail
        pending_tail = None
        # I10: w = p - n (bf16 out over pt, in-place with in0)
        wf = pt[:].bitcast(dt.bfloat16)
        nc.vector.tensor_tensor(out=wf, in0=pt[:], in1=x16[:], op=op.subtract)
        # I11: q = w * delta -> qt (t dead after I9)
        ddb = dd[:].bitcast(dt.bfloat16)[:, None, :].broadcast_to([128, 2, F2])
        nc.vector.tensor_tensor(
            out=qt[:].rearrange("p (r f) -> p r f", r=2),
            in0=wf.rearrange("p (r f) -> p r f", r=2),
            in1=ddb, op=op.mult,
        )
        pending_out = (bn, bs, qt)

    for nn in range(NPC * S * R):
        n = (nn // S) % NPC
        s = nn % S
        hs = slice(s * F2, (s + 1) * F2)
        xt = big.tile([128, 2 * F2], dt.float32, tag="xt")
        nc.sync.dma_start(out=xt[:, 0:F2], in_=x_d[n, 0:128, hs])
        nc.sync.dma_start(out=xt[:, F2 : 2 * F2], in_=x_d[n, 128:256, hs])

        # I1: x16 = high_halves | 1 (odd mantissa; cannot change exponents)
        x16 = deep.tile([128, 2 * F2], dt.int16, tag="x16")
        xhi = xt[:].bitcast(dt.int16).rearrange(
            "p (f two) -> p f two", two=2
        )[:, :, 1]
        nc.vector.tensor_scalar(
            out=x16[:], in0=xhi, scalar1=1, scalar2=None, op0=op.bitwise_or
        )
        # I2: exf -> pt slot (pt is free until I8)
        pt = midp.tile([128, 2 * F2], dt.int16, tag="pt")
        nc.vector.tensor_scalar(
            out=pt[:], in0=x16[:], scalar1=0x7F80, scalar2=None,
            op0=op.bitwise_and,
        )
        # I3: merge the two C halves -> staged in xt's i16 view (xt dead)
        m = xt[:].bitcast(dt.int16)[:, 0:F2]
        nc.vector.tensor_tensor(
            out=m, in0=pt[:, 0:F2], in1=pt[:, F2 : 2 * F2], op=op.max
        )
        # I4: cross-partition max -> block exponent field, broadcast
        mx = small.tile([128, F2], dt.int16, tag="mx")
        nc.gpsimd.partition_all_reduce(mx[:], m, 128, bass_isa.ReduceOp.max)
        # out-DMA of the sub-image two back: queued on gpsimd AFTER this
        # reduce, input long ready -> no queue stall
        flush_out()
        # mid of sub-image k-1 (its reduce completed during this frontend),
        # then tail of k-2 (its ScalarE completed during iteration k-1)
        emit_mid()
        emit_tail()
        pending_mid = (n, s, x16, pt, mx)
    # epilogue: drain the pipeline; flush pending_out before any stage
    # that would overwrite it
    flush_out()      # out-DMA(last-2)
    emit_tail()      # tail(last-1) -> pending_out
    flush_out()      # out-DMA(last-1)
    emit_mid()       # mid(last)
    emit_tail()      # tail(last) -> pending_out
    flush_out()      # out-DMA(last)


def _v5_body(nc, big, small, negcm, x_d, y_d, R, variant):
    import concourse.mybir as mybir
    from concourse import bass_isa

    dt = mybir.dt
    op = mybir.AluOpType
    pending_out = None
    if True:  # preserve indentation of the original loop
            for nn in range(NPC * R):
                n = nn % NPC
                xt = big.tile([128, 2 * F], dt.float32, tag="xt")
                if variant == "onedma":
                    nc.sync.dma_start(
                        out=xt[:].rearrange("p (r f) -> p r f", r=2),
                        in_=x_d[n].rearrange("(r p) f -> p r f", p=128),
                    )
                else:
                    nc.sync.dma_start(out=xt[:, 0:F], in_=x_d[n, 0:128, :])
                    nc.sync.dma_start(out=xt[:, F : 2 * F], in_=x_d[n, 128:256, :])

                # I1: high halves (odd int16 lanes of the f32 words), odd mantissa
                x16 = big.tile([128, 2 * F], dt.int16, tag="x16")
                xhi = xt[:].bitcast(dt.int16).rearrange(
                    "p (f two) -> p f two", two=2
                )[:, :, 1]
                eng1 = nc.gpsimd if variant == "i1gp" else nc.vector
                eng1.tensor_scalar(
                    out=x16[:], in0=xhi,
                    scalar1=1, scalar2=None, op0=op.bitwise_or,
                )
                # I2: exponent fields
                exf = big.tile([128, 2 * F], dt.int16, tag="exf")
                nc.vector.tensor_scalar(
                    out=exf[:], in0=x16[:],
                    scalar1=0x7F80, scalar2=None, op0=op.bitwise_and,
                )
                # I3: merge the two C halves
                m = small.tile([128, F], dt.int16, tag="m")
                nc.vector.tensor_tensor(
                    out=m[:], in0=exf[:, 0:F], in1=exf[:, F : 2 * F], op=op.max
                )
                # I4: cross-partition max -> block exponent field, broadcast
                if variant == "noreduce":
                    mx = m
                elif variant == "redhalf":
                    # timing probe: reduce only half the columns (wrong results)
                    mx = small.tile([128, F], dt.int16, tag="mx")
                    nc.gpsimd.partition_all_reduce(
                        mx[:, 0 : F // 2], m[:, 0 : F // 2],
                        128, bass_isa.ReduceOp.max,
                    )
                    nc.vector.tensor_scalar(
                        out=mx[:, F // 2 : F], in0=m[:, F // 2 : F],
                        scalar1=0, scalar2=None, op0=op.bitwise_or,
                    )
                elif variant == "red8":
                    # E-128 fits int8 (E = eb>>7 in [1, 254])
                    m8 = small.tile([128, F], dt.int8, tag="m8")
                    nc.vector.tensor_scalar(
                        out=m8[:], in0=m[:],
                        scalar1=0.0078125, scalar2=-128, op0=op.mult, op1=op.add,
                    )
                    mx8 = small.tile([128, F], dt.int8, tag="mx8")
                    nc.gpsimd.partition_all_reduce(
                        mx8[:], m8[:], 128, bass_isa.ReduceOp.max
                    )
                else:
                    mx = small.tile([128, F], dt.int16, tag="mx")
                    nc.gpsimd.partition_all_reduce(
                        mx[:], m[:], 128, bass_isa.ReduceOp.max
                    )
                if variant == "lateout" and pending_out is not None:
                    # emit the PREVIOUS image's out-DMA after this reduce so
                    # the gpsimd queue never waits on the DVE backend
                    pn, pq = pending_out
                    nc.gpsimd.dma_start(out=y_d[pn, 0:128, :], in_=pq[:, 0:F])
                    nc.gpsimd.dma_start(
                        out=y_d[pn, 128:256, :], in_=pq[:, F : 2 * F]
                    )
                    pending_out = None
                # I5: bits(-1/delta) = -eb (mod 2^16)
                rn = small.tile([128, F], dt.int16, tag="rn")
                # I6: bits(delta) = eb - 0x0100
                dd = small.tile([128, F], dt.int16, tag="dd")
                if variant == "red8":
                    # eb = (mx8+128)*128; rn = -eb; dd = eb - 0x0100
                    nc.vector.tensor_scalar(
                        out=rn[:], in0=mx8[:],
                        scalar1=-128, scalar2=-16384, op0=op.mult, op1=op.add,
                    )
                    nc.vector.tensor_scalar(
                        out=dd[:], in0=mx8[:],
                        scalar1=128, scalar2=16128, op0=op.mult, op1=op.add,
                    )
                else:
                    nc.vector.tensor_scalar(
                        out=rn[:], in0=mx[:],
                        scalar1=-1, scalar2=None, op0=op.mult,
                    )
                    nc.vector.tensor_scalar(
                        out=dd[:], in0=mx[:],
                        scalar1=-0x0100, scalar2=None, op0=op.add,
                    )
                # I7: t = x' * (-1/delta)   (bf16, exact)
                t = exf  # reuse slot (exf dead after I3)
                x3 = x16[:].bitcast(dt.bfloat16).rearrange("p (r f) -> p r f", r=2)
                rnb = rn[:].bitcast(dt.bfloat16)[:, None, :].broadcast_to([128, 2, F])
                nc.vector.tensor_tensor(
                    out=t[:].bitcast(dt.bfloat16).rearrange("p (r f) -> p r f", r=2),
                    in0=x3, in1=rnb, op=op.mult,
                )
                tb = t[:].bitcast(dt.bfloat16)
                # I8/I9: trunc via Relu pair + RN converters (ScalarE)
                if variant == "noscalar":
                    p16 = tb  # timing-only: skip ScalarE, keep DVE shape
                    n16 = x16[:].bitcast(dt.bfloat16)
                else:
                    pt = big.tile([128, 2 * F], dt.int16, tag="pt")
                    p16 = pt[:]
                    n16 = x16[:]  # x16 dead after I7
                    nc.scalar.activation(
                        out=p16, in_=tb,
                        func=mybir.ActivationFunctionType.Relu,
                        bias=negcm[:], scale=-1.0,
                    )
                    nc.scalar.activation(
                        out=n16, in_=tb,
                        func=mybir.ActivationFunctionType.Relu,
                        bias=negcm[:], scale=1.0,
                    )
                # I10: w = p - n, cast to bf16 (arith op may cast; |w| <= 7)
                wf = exf[:].bitcast(dt.bfloat16)  # t dead after I8/I9
                nc.vector.tensor_tensor(out=wf, in0=p16, in1=n16, op=op.subtract)
                # I11: q = w * delta  (bf16 * bf16 -> bf16, exact)
                q = small.tile([128, 2 * F], dt.bfloat16, tag="q")
                ddb = dd[:].bitcast(dt.bfloat16)[:, None, :].broadcast_to([128, 2, F])
                nc.vector.tensor_tensor(
                    out=q[:].rearrange("p (r f) -> p r f", r=2),
                    in0=wf.rearrange("p (r f) -> p r f", r=2),
                    in1=ddb, op=op.mult,
                )
                # out: SWDGE DMA with bf16 -> f32 cast
                if variant == "lateout":
                    pending_out = (n, q)
                elif variant == "onedma":
                    nc.gpsimd.dma_start(
                        out=y_d[n].rearrange("(r p) f -> p r f", p=128),
                        in_=q[:].rearrange("p (r f) -> p r f", r=2),
                    )
                elif variant != "noout":
                    nc.gpsimd.dma_start(out=y_d[n, 0:128, :], in_=q[:, 0:F])
                    nc.gpsimd.dma_start(
                        out=y_d[n, 128:256, :], in_=q[:, F : 2 * F]
                    )
            if pending_out is not None:
                pn, pq = pending_out
                nc.gpsimd.dma_start(out=y_d[pn, 0:128, :], in_=pq[:, 0:F])
                nc.gpsimd.dma_start(
                    out=y_d[pn, 128:256, :], in_=pq[:, F : 2 * F]
                )


def _hwout_body(nc, big, small, negcm, zerob, x_d, y_d, R):
    """v6: GPSIMD runs ONLY the partition reduce; output path is ScalarE
    widen (bf16 -> f32, exact) + HWDGE (sync) DMA, so the gpsimd queue never
    waits on the DVE backend between reduces. Heavy in-place slot reuse to
    fit 2x buffering in SBUF.

    Per image: xt(input, dies I1) -> x16(x'|1, becomes n16) ->
    pt(exf -> p16 -> w) -> qt(t -> q bf16) -> q32 -> sync DMA.
    m tile becomes dd; mx tile becomes rn (in-place negate).
    """
    import concourse.mybir as mybir
    from concourse import bass_isa

    dt = mybir.dt
    op = mybir.AluOpType
    for nn in range(NPC * R):
        n = nn % NPC
        xt = big.tile([128, 2 * F], dt.float32, tag="xt")
        nc.sync.dma_start(out=xt[:, 0:F], in_=x_d[n, 0:128, :])
        nc.sync.dma_start(out=xt[:, F : 2 * F], in_=x_d[n, 128:256, :])

        # I1: x16 = high_halves | 1
        x16 = big.tile([128, 2 * F], dt.int16, tag="x16")
        xhi = xt[:].bitcast(dt.int16).rearrange(
            "p (f two) -> p f two", two=2
        )[:, :, 1]
        nc.vector.tensor_scalar(
            out=x16[:], in0=xhi, scalar1=1, scalar2=None, op0=op.bitwise_or
        )
        # I2: exf -> pt slot
        pt = big.tile([128, 2 * F], dt.int16, tag="pt")
        nc.vector.tensor_scalar(
            out=pt[:], in0=x16[:], scalar1=0x7F80, scalar2=None,
            op0=op.bitwise_and,
        )
        # I3: merge halves
        m = small.tile([128, F], dt.int16, tag="m")
        nc.vector.tensor_tensor(
            out=m[:], in0=pt[:, 0:F], in1=pt[:, F : 2 * F], op=op.max
        )
        # I4: cross-partition max (gpsimd, the only gpsimd user)
        mx = small.tile([128, F], dt.int16, tag="mx")
        nc.gpsimd.partition_all_reduce(mx[:], m[:], 128, bass_isa.ReduceOp.max)
        # I6: dd = eb - 0x0100 -> m slot (m dead)
        dd = m
        nc.vector.tensor_scalar(
            out=dd[:], in0=mx[:], scalar1=-0x0100, scalar2=None, op0=op.add
        )
        # I5: rn = -eb, in-place over mx (AFTER dd)
        rn = mx
        nc.vector.tensor_scalar(
            out=rn[:], in0=mx[:], scalar1=-1, scalar2=None, op0=op.mult
        )
        # I7: t = x' * (-1/delta) -> qt
        qt = small.tile([128, 2 * F], dt.bfloat16, tag="qt")
        x3 = x16[:].bitcast(dt.bfloat16).rearrange("p (r f) -> p r f", r=2)
        rnb = rn[:].bitcast(dt.bfloat16)[:, None, :].broadcast_to([128, 2, F])
        nc.vector.tensor_tensor(
            out=qt[:].rearrange("p (r f) -> p r f", r=2),
            in0=x3, in1=rnb, op=op.mult,
        )
        tb = qt[:]
        # I8/I9: Relu pair + RN cvt (ScalarE); p -> pt (exf dead),
        # n -> x16 (x' dead after I7)
        p16 = pt[:]
        n16 = x16[:]
        nc.scalar.activation(
            out=p16, in_=tb, func=mybir.ActivationFunctionType.Relu,
            bias=negcm[:], scale=-1.0,
        )
        nc.scalar.activation(
            out=n16, in_=tb, func=mybir.ActivationFunctionType.Relu,
            bias=negcm[:], scale=1.0,
        )
        # I10: w = p - n (bf16 view over pt, in-place with in0)
        wf = pt[:].bitcast(dt.bfloat16)
        nc.vector.tensor_tensor(out=wf, in0=p16, in1=n16, op=op.subtract)
        # I11: q = w * delta -> qt (t dead after I9)
        ddb = dd[:].bitcast(dt.bfloat16)[:, None, :].broadcast_to([128, 2, F])
        nc.vector.tensor_tensor(
            out=qt[:].rearrange("p (r f) -> p r f", r=2),
            in0=wf.rearrange("p (r f) -> p r f", r=2),
            in1=ddb, op=op.mult,
        )
        # widen bf16 -> f32 on ScalarE (exact), then HWDGE DMA out
        q32 = big.tile([128, 2 * F], dt.float32, tag="q32")
        nc.scalar.activation(
            out=q32[:], in_=qt[:],
            func=mybir.ActivationFunctionType.Identity,
            bias=zerob[:], scale=1.0,
        )
        nc.sync.dma_start(out=y_d[n, 0:128, :], in_=q32[:, 0:F])
        nc.sync.dma_start(out=y_d[n, 128:256, :], in_=q32[:, F : 2 * F])


def _run(x, trace=False, **kwargs):
    from concourse import bass_utils

    nc = _build()
    xs = np.ascontiguousarray(x.reshape(N, C, HW))
    in_maps = [
        {"x": xs[i * NPC : (i + 1) * NPC]} for i in range(NCORES)
    ]
    res = bass_utils.run_bass_kernel_spmd(
        nc, in_maps, core_ids=list(range(NCORES)), trace=trace, **kwargs
    )
    out = np.concatenate([r["y"] for r in res.results], axis=0)
    return out.reshape(N, C, H, W), res


def kernel(activations):
    out, _ = _run(np.asarray(activations))
    return out
